# revision 16
# baseline (speedup 1.0000x reference)
"""GQA kernel for Trainium2, 8 NeuronCores.

Problem: B=2, T=2048, D=2048, 16 query heads / 2 KV heads, d_head=128, causal.

Sharding: core c -> batch b = c//4, head-quarter q = c%4 (query heads
4q..4q+3, kv head q//2). Each core computes its 4 heads' attention and a
partial output projection (its Wo rows); host sums the 4 partials per batch
and adds bo.

Host marshalling: weights and x are pre-cast to bf16 and pre-tiled so each
logical input lands with ONE multi-dim DMA per priority chunk:
  xt [128, 16, 2048] = x[b].T tiled as (p, kb, t)
  wq [128, 16, 512], wk/wv [128, 16, 128]  (p, kb, cols)
  wo [128, 4, 2048]                        (p=dh, h, n)
DMA priority order (two queues, sync+gpsimd, halves of each chunk):
biases, wk, wv, x slice0, wq, wo, x slices 1-3 -- so PE starts ~9us in and
never starves.

On-core dataflow (bf16 matmuls, fp32 PSUM):
per 512-wide t-slice j: K/V/Q projections (ACT epilogues w/ bias);
V PE-transposed to natural layout; attention per head h over tk blocks,
one [128,512] PSUM bank per S^T block (exp on ACT, never bank-crossing),
causal diagonal blocks column-trimmed (S/exp/PV/racc only touch the valid
tq range, stored at column 0; 128x128 affine_select masks on gpsimd only
for the true diagonal), PV accumulated into otps; softmax denominator:
bf16 pair-tree + fp32 racc on DVE, bf16 cast, one ones-matmul partition
reduction per (h,j); normalize (reciprocal+mul) on DVE.
Output projection for slice j is cut into 4-matmul units used as PE filler
inside slice j+1's attention rounds (keeps PE fed while ACT exp runs);
PSUM->SBUF staging copies split DVE/ACT; output DMA on the scalar queue.
"""

import numpy as np
import ml_dtypes
from contextlib import ExitStack

import concourse.bass as bass
from concourse import bacc
import concourse.mybir as mybir
import concourse.tile as tile
from concourse.bass_utils import run_bass_kernel_spmd
from concourse.masks import make_identity

F32 = mybir.dt.float32
F32R = mybir.dt.float32r
BF16 = mybir.dt.bfloat16

D = 2048
T = 2048
DH = 128
B = 2
HPC = 4            # query heads per core
NCORES = 8
SCALE = 1.0 / float(np.sqrt(128.0))

_CACHE = {}


def _build_nc():
    nc = bacc.Bacc("TRN2", target_bir_lowering=False, debug=False,
                   num_devices=NCORES)

    xt = nc.dram_tensor("xt", [128, 16, T], BF16, kind="ExternalInput")
    wq = nc.dram_tensor("wq", [128, 16, HPC * DH], BF16, kind="ExternalInput")
    wk = nc.dram_tensor("wk", [128, 16, DH], BF16, kind="ExternalInput")
    wv = nc.dram_tensor("wv", [128, 16, DH], BF16, kind="ExternalInput")
    wo = nc.dram_tensor("wo", [128, HPC, D], BF16, kind="ExternalInput")
    bqm = nc.dram_tensor("bqm", [DH, HPC], F32, kind="ExternalInput")
    bkm = nc.dram_tensor("bkm", [DH, 1], F32, kind="ExternalInput")
    bvm = nc.dram_tensor("bvm", [DH, 1], F32, kind="ExternalInput")
    part = nc.dram_tensor("part", [T, D], F32, kind="ExternalOutput")

    with ExitStack() as ctx:
        tc = ctx.enter_context(tile.TileContext(nc))
        persist = ctx.enter_context(tc.tile_pool(name="persist", bufs=1))
        work = ctx.enter_context(tc.tile_pool(name="work", bufs=2))
        psum = ctx.enter_context(tc.tile_pool(name="psum", bufs=2, space="PSUM"))

        # ---- constants ----
        ones32 = persist.tile([128, 128], BF16, tag="ones32", name="ones32")
        nc.vector.memset(ones32, 1.0)
        ident = persist.tile([128, 128], BF16, tag="ident", name="ident")
        make_identity(nc, ident)
        # warm the ACT exp table-set (~2.7us) during the initial DMA wait
        warm = persist.tile([128, 1], F32, tag="warm", name="warm")
        nc.scalar.activation(out=warm, in_=ident[:, 0:1],
                             func=mybir.ActivationFunctionType.Exp)

        bq_sb = persist.tile([DH, HPC], F32, tag="bq", name="bq_sb")
        bk_sb = persist.tile([DH, 1], F32, tag="bk", name="bk_sb")
        bv_sb = persist.tile([DH, 1], F32, tag="bv", name="bv_sb")

        # ---- persistent input tiles ----
        x_all = persist.tile([128, 16, T], BF16, tag="x_all", name="x_all")
        wq_sb = persist.tile([128, 16, HPC * DH], BF16, tag="wq", name="wq_sb")
        wk_sb = persist.tile([128, 16, DH], BF16, tag="wk", name="wk_sb")
        wv_sb = persist.tile([128, 16, DH], BF16, tag="wv", name="wv_sb")
        wo_sb = persist.tile([128, HPC, D], BF16, tag="wo", name="wo_sb")

        # ---- input DMAs in consumption order on three queues ----
        # kb-quad granularity for wk/wv/x-slice0 so the projection matmuls
        # (which accumulate kb in order) can start as soon as the first
        # quads land; startup chunks spread over sync/scalar/gpsimd queues.
        def quad_dma(e, quad):
            kbs = slice(4 * quad, 4 * quad + 4)
            e.dma_start(out=wk_sb[:, kbs, :], in_=wk[:, kbs, :])
            e.dma_start(out=wv_sb[:, kbs, :], in_=wv[:, kbs, :])
            e.dma_start(out=x_all[:, kbs, 0:512], in_=xt[:, kbs, 0:512])

        def wq_dma(e, h):
            cs = slice(h * 128, (h + 1) * 128)
            e.dma_start(out=wq_sb[:, :, cs], in_=wq[:, :, cs])

        nc.scalar.dma_start(out=bq_sb, in_=bqm[:, :])
        nc.scalar.dma_start(out=bk_sb, in_=bkm[:, :])
        nc.scalar.dma_start(out=bv_sb, in_=bvm[:, :])
        quad_dma(nc.sync, 0)
        quad_dma(nc.scalar, 1)
        quad_dma(nc.gpsimd, 3)
        wq_dma(nc.gpsimd, 0)
        quad_dma(nc.sync, 2)
        wq_dma(nc.scalar, 1)
        wq_dma(nc.sync, 2)
        wq_dma(nc.gpsimd, 3)
        nc.scalar.dma_start(out=wo_sb[:, 0:2, :], in_=wo[:, 0:2, :])
        nc.gpsimd.dma_start(out=wo_sb[:, 2:4, :], in_=wo[:, 2:4, :])
        for js in range(1, 4):
            sl = slice(js * 512, (js + 1) * 512)
            nc.sync.dma_start(out=x_all[:, 0:8, sl], in_=xt[:, 0:8, sl])
            nc.gpsimd.dma_start(out=x_all[:, 8:16, sl], in_=xt[:, 8:16, sl])

        # ---- persistent activations ----
        kT = persist.tile([128, T], BF16, tag="kT", name="kT")
        v_sb = [persist.tile([128, DH], BF16, tag=f"v{t}", name=f"v{t}")
                for t in range(16)]
        # per-slice q and o (o double-buffered: outproj(j) runs during j+1)
        qT = [persist.tile([128, 512], BF16, tag=f"qT{h}", name=f"qT{h}")
              for h in range(HPC)]
        oT = [[persist.tile([128, 512], BF16, tag=f"oT{d}_{h}",
                            name=f"oT{d}_{h}")
               for h in range(HPC)] for d in range(2)]

        # ---------- filler machinery ----------
        # Each filler unit is a closure emitting a few PE matmuls (+ epilogue
        # ops on other engines).  Units are popped inside attention pair
        # rounds to keep PE busy while ACT computes exp.
        fillers = []

        def pop_fillers(k):
            for _ in range(min(k, len(fillers))):
                fillers.pop(0)()

        def drain_fillers():
            while fillers:
                fillers.pop(0)()

        # ---------- projection helpers ----------
        def qproj_quarter(j, h, qps, kq):
            def emit():
                for kb in range(4 * kq, 4 * kq + 4):
                    nc.tensor.matmul(out=qps,
                                     lhsT=wq_sb[:, kb, h * 128:(h + 1) * 128],
                                     rhs=x_all[:, kb, j * 512:(j + 1) * 512],
                                     start=(kb == 0), stop=(kb == 15))
                if kq == 3:
                    nc.scalar.activation(out=qT[h], in_=qps,
                                         func=mybir.ActivationFunctionType.Identity,
                                         bias=bq_sb[:, h:h + 1], scale=1.0)
            return emit

        def emit_qproj(j, h):
            qps = psum.tile([128, 512], F32, tag="acc", bufs=2,
                            name=f"qps{j}_{h}")
            for kq in range(4):
                qproj_quarter(j, h, qps, kq)()

        KB_ORDER = [0, 1, 2, 3, 4, 5, 6, 7, 12, 13, 14, 15, 8, 9, 10, 11]

        def emit_kvproj(j):
            sl = slice(j * 512, (j + 1) * 512)
            kps = psum.tile([128, 512], F32, tag="acc", bufs=2, name=f"kps{j}")
            for i, kb in enumerate(KB_ORDER):
                nc.tensor.matmul(out=kps, lhsT=wk_sb[:, kb, :],
                                 rhs=x_all[:, kb, sl],
                                 start=(i == 0), stop=(i == 15))
            nc.scalar.activation(out=kT[:, sl], in_=kps,
                                 func=mybir.ActivationFunctionType.Identity,
                                 bias=bk_sb[:, 0:1], scale=1.0)
            vps = psum.tile([128, 512], F32, tag="acc", bufs=2, name=f"vps{j}")
            for i, kb in enumerate(KB_ORDER):
                nc.tensor.matmul(out=vps, lhsT=wv_sb[:, kb, :],
                                 rhs=x_all[:, kb, sl],
                                 start=(i == 0), stop=(i == 15))
            vt_sb = work.tile([128, 512], BF16, tag="vt", bufs=2,
                              name=f"vt{j}")
            nc.scalar.activation(out=vt_sb, in_=vps,
                                 func=mybir.ActivationFunctionType.Identity,
                                 bias=bv_sb[:, 0:1], scale=1.0)
            vtp = psum.tile([128, 512], BF16, tag="op", bufs=3, name=f"vtp{j}")
            for sub in range(4):
                nc.tensor.transpose(vtp[:, sub * 128:(sub + 1) * 128],
                                    vt_sb[:, sub * 128:(sub + 1) * 128],
                                    ident)
            for sub in range(4):
                nc.vector.tensor_copy(out=v_sb[4 * j + sub],
                                      in_=vtp[:, sub * 128:(sub + 1) * 128])

        # ---------- output projection units (filler fodder) ----------
        _ostg = {}

        def outproj_unit(j, tt, n, last_tt):
            # 4 matmuls (contract heads) + staging copy (+ DMA when ready)
            def emit():
                if tt not in _ostg:
                    _ostg[tt] = work.tile([128, D], F32, tag="ostg", bufs=2,
                                          name=f"ostg{tt}")
                ostg = _ostg[tt]
                ops = psum.tile([128, 512], F32, tag="op", bufs=3,
                                name=f"ops{tt}_{n}")
                sub = tt % 4
                for h in range(HPC):
                    nc.tensor.matmul(
                        out=ops,
                        lhsT=oT[j % 2][h][:, sub * 128:(sub + 1) * 128],
                        rhs=wo_sb[:, h, n * 512:(n + 1) * 512],
                        start=(h == 0), stop=(h == HPC - 1))
                if n % 2 == 0:
                    nc.vector.tensor_copy(
                        out=ostg[:, n * 512:(n + 1) * 512], in_=ops)
                else:
                    nc.scalar.copy(out=ostg[:, n * 512:(n + 1) * 512],
                                   in_=ops)
                if last_tt:
                    # per-unit DMA to shorten the kernel tail
                    nc.scalar.dma_start(
                        out=part[tt * 128:(tt + 1) * 128,
                                 n * 512:(n + 1) * 512],
                        in_=ostg[:, n * 512:(n + 1) * 512])
                elif n == 3:
                    nc.scalar.dma_start(
                        out=part[tt * 128:(tt + 1) * 128, :], in_=ostg)
            return emit

        def queue_outproj(j):
            for sub in range(4):
                tt = 4 * j + sub
                for n in range(4):
                    fillers.append(
                        outproj_unit(j, tt, n, last_tt=(tt >= 12)))

        # ---------- attention ----------
        def emit_attention_head(j, h, fill_rate):
            """Attention for head h over tq-slice j, tk blocks 0..4j+3.
            One [128,512] PSUM bank per S^T block (exp never crosses banks,
            matmul outs always bank-aligned); the 4 diagonal blocks are
            column-trimmed: block r computes only tq in [128r, 512), stored
            at column 0 of its tile.  Softmax denominator accumulates via a
            bf16 pair tree on DVE."""
            otps = psum.tile([128, 512], F32, tag="acc", bufs=2,
                             name=f"otps{h}_{j}")
            racc = work.tile([128, 512], F32, tag="racc", bufs=2,
                             name=f"racc{h}_{j}")

            def sp_block(tkb, w_off):
                """S^T block for tk tile tkb covering tq [w_off:512), then
                exp -> pt[:, 0:512-w_off]."""
                sps = psum.tile([128, 512], F32, tag="sps", bufs=3,
                                name=f"sps{h}_{j}_{tkb}")
                w = 512 - w_off
                nc.tensor.matmul(
                    out=sps[:, 0:w],
                    lhsT=kT[:, tkb * 128:(tkb + 1) * 128],
                    rhs=qT[h][:, w_off:512],
                    start=True, stop=True)
                pt = work.tile([128, 512], BF16, tag="pt", bufs=4,
                               name=f"pt{h}_{j}_{tkb}")
                nc.scalar.activation(out=pt[:, 0:w], in_=sps[:, 0:w],
                                     func=mybir.ActivationFunctionType.Exp,
                                     scale=SCALE)
                return pt

            def pv_mm(tkb, pt_ap, o_off, start, stop):
                nc.tensor.matmul(out=otps[:, o_off:512], lhsT=v_sb[tkb],
                                 rhs=pt_ap, start=start, stop=stop,
                                 skip_group_check=True)

            # --- non-diagonal blocks ---
            # denominator tree: bf16 pair sums on gpsimd, bf16 quad sums on
            # DVE (2x mode), fp32 spine on DVE -- spreads the reduction so
            # neither DVE nor gpsimd binds in late slices.
            pps = []
            racc_init = [False]

            def racc_accum(ap):
                if not racc_init[0]:
                    nc.vector.tensor_copy(out=racc, in_=ap)
                    racc_init[0] = True
                else:
                    nc.vector.tensor_add(out=racc, in0=racc, in1=ap)

            for p in range(2 * j):
                pt0 = sp_block(2 * p, 0)
                pv_mm(2 * p, pt0, 0, start=(p == 0), stop=False)
                pt1 = sp_block(2 * p + 1, 0)
                pv_mm(2 * p + 1, pt1, 0, start=False, stop=False)
                pp = work.tile([128, 512], BF16, tag="ppair", bufs=4,
                               name=f"pp{h}_{j}_{p}")
                nc.vector.tensor_add(out=pp, in0=pt0, in1=pt1)
                pps.append(pp)
                if len(pps) == 2:
                    qs = work.tile([128, 512], BF16, tag="qsum", bufs=2,
                                   name=f"qs{h}_{j}_{p}")
                    nc.vector.tensor_add(out=qs, in0=pps[0], in1=pps[1])
                    pps.clear()
                    racc_accum(qs)
                pop_fillers(fill_rate)
            if pps:
                racc_accum(pps[0])
                pps.clear()

            # --- diagonal blocks r=0..3, column-trimmed ---
            base = 4 * j
            for r in range(4):
                w_off = 128 * r
                w = 512 - w_off
                pt = sp_block(base + r, w_off)
                nc.gpsimd.affine_select(
                    out=pt[:, 0:128], in_=pt[:, 0:128],
                    compare_op=mybir.AluOpType.is_ge,
                    fill=0.0, base=0,
                    pattern=[[1, 128]], channel_multiplier=-1)
                pv_mm(base + r, pt[:, 0:w], w_off,
                      start=(j == 0 and r == 0), stop=(r == 3))
                if r == 0:
                    racc_accum(pt)
                else:
                    nc.vector.tensor_add(out=racc[:, w_off:512],
                                         in0=racc[:, w_off:512],
                                         in1=pt[:, 0:w])
                pop_fillers(fill_rate)

            # --- denominator: bf16 ones-matmul partition reduction ---
            racc16 = work.tile([128, 512], BF16, tag="racc16", bufs=2,
                               name=f"racc16{h}_{j}")
            nc.vector.tensor_copy(out=racc16, in_=racc)
            rsb = psum.tile([128, 512], F32, tag="op", bufs=3,
                            name=f"rsb{h}_{j}")
            nc.tensor.matmul(out=rsb, lhsT=ones32, rhs=racc16,
                             start=True, stop=True)
            rinv = work.tile([128, 512], F32, tag="rinv", bufs=2,
                             name=f"rinv{h}_{j}")
            nc.vector.reciprocal_approx_fast(rinv, rsb)
            nc.vector.tensor_mul(out=oT[j % 2][h], in0=otps, in1=rinv)

        # ---------- main schedule ----------
        for j in range(4):
            emit_kvproj(j)
            if j == 0:
                emit_qproj(0, 0)
                emit_qproj(0, 1)
                qps2 = psum.tile([128, 512], F32, tag="acc", bufs=2,
                                 name="qps0_2f")
                fillers.extend(qproj_quarter(0, 2, qps2, kq)
                               for kq in range(4))
                emit_attention_head(0, 0, fill_rate=2)
                drain_fillers()
                qps3 = psum.tile([128, 512], F32, tag="acc", bufs=2,
                                 name="qps0_3f")
                fillers.extend(qproj_quarter(0, 3, qps3, kq)
                               for kq in range(4))
                emit_attention_head(0, 1, fill_rate=2)
                drain_fillers()
                emit_attention_head(0, 2, fill_rate=0)
                emit_attention_head(0, 3, fill_rate=0)
            else:
                for h in range(HPC):
                    emit_qproj(j, h)
                rate = 1
                for h in range(HPC):
                    emit_attention_head(j, h, fill_rate=rate)
            queue_outproj(j)
            if j > 0:
                # keep at most one slice's worth of units pending
                while len(fillers) > 16:
                    fillers.pop(0)()
        drain_fillers()

    nc.compile()
    return nc


def _get_nc():
    if "nc" not in _CACHE:
        _CACHE["nc"] = _build_nc()
    return _CACHE["nc"]


def _bf16(a):
    return np.ascontiguousarray(a.astype(ml_dtypes.bfloat16))


def _tile16(a):
    # [2048, C] -> [128, 16, C]   (rows kb*128+p -> [p, kb, :])
    c = a.shape[1]
    return np.ascontiguousarray(
        a.reshape(16, 128, c).transpose(1, 0, 2))


def kernel(x, Wq, bq, Wk, bk, Wv, bv, Wo, bo, **kw):
    x = np.asarray(x, dtype=np.float32)
    Wq = np.asarray(Wq, dtype=np.float32)
    Wk = np.asarray(Wk, dtype=np.float32)
    Wv = np.asarray(Wv, dtype=np.float32)
    Wo = np.asarray(Wo, dtype=np.float32)
    bq = np.asarray(bq, dtype=np.float32)
    bk = np.asarray(bk, dtype=np.float32)
    bv = np.asarray(bv, dtype=np.float32)
    bo = np.asarray(bo, dtype=np.float32)

    nc = _get_nc()
    xt_b = [_tile16(_bf16(x[b].T)) for b in range(B)]
    in_maps = []
    for c in range(NCORES):
        b = c // 4
        q = c % 4
        hs = q * HPC * DH          # column start in Wq / row start in Wo
        kv = q // 2
        bq_m = np.ascontiguousarray(
            bq[hs:hs + HPC * DH].reshape(HPC, DH).T)          # [128, 4]
        bk_m = np.ascontiguousarray(
            bk[kv * DH:(kv + 1) * DH].reshape(DH, 1))         # [128, 1]
        bv_m = np.ascontiguousarray(
            bv[kv * DH:(kv + 1) * DH].reshape(DH, 1))         # [128, 1]
        in_maps.append({
            "xt": xt_b[b],
            "wq": _tile16(_bf16(Wq[:, hs:hs + HPC * DH])),
            "wk": _tile16(_bf16(Wk[:, kv * DH:(kv + 1) * DH])),
            "wv": _tile16(_bf16(Wv[:, kv * DH:(kv + 1) * DH])),
            "wo": np.ascontiguousarray(
                _bf16(Wo[hs:hs + HPC * DH, :]).reshape(HPC, 128, D)
                .transpose(1, 0, 2)),
            "bqm": bq_m,
            "bkm": bk_m,
            "bvm": bv_m,
        })

    res = run_bass_kernel_spmd(nc, in_maps, list(range(NCORES)),
                               **kw.get("_run_kwargs", {}))
    if kw.get("_return_res"):
        return res
    parts = [res.results[c]["part"] for c in range(NCORES)]
    out = np.empty((B, T, D), dtype=np.float32)
    for b in range(B):
        acc = parts[4 * b].astype(np.float32).copy()
        for q in range(1, 4):
            acc += parts[4 * b + q]
        out[b] = acc + bo[None, :]
    return out
